# revision 4
# baseline (speedup 1.0000x reference)
"""Trainium2 Bass kernel for relational GNN message passing (8 NeuronCores).

Math (per relation r, adjacency A_r from COO edge lists, X = features):
    out  = diag(1/full_deg) * sum_r [ (A_r X) W_r + deg_r*bias_r
                                      + (A_r^T X) Wt_r + deg_t_r*bias_t_r ]

Strategy (row-parallel over output nodes, hinted sharding):
  - 8 "virtual relations" k = (r, direction); output row o and gather row g
    per edge.  Nodes sharded 6250/core (padded 6272 = 49 blocks of 128).
  - Host prep (integer bookkeeping only): bucket edges by (core, k, block),
    pad each bucket to chunks of 128 edge slots; per-slot gather index
    (int32) and local-row id (fp32).  Degree counts (bincount) are shipped
    as a [8 x rows] matrix; deg*bias is computed ON DEVICE via a K=8 matmul.
  - Device per (k, block): indirect-DMA gather of 128 X-rows/chunk (bf16),
    selector S[e, row] = (id_e == row) built by DVE is_equal against an
    iota constant, then PE computes axT pieces: G_f^T @ S accumulated in
    PSUM (f = 4 feature pieces of 128).  Dense stage: out += axT_f^T @ W_kf
    accumulated across k in PSUM, + deg-bias matmul, then scaled by
    1/full_deg (per-partition scalar) and DMA'd out.
  - No collectives: every core owns disjoint output rows; X is replicated.

kernel(**inputs) takes FULL inputs and returns the FULL [50000, 512] fp32
output; it shards, compiles, runs on cores 0-7 via run_bass_kernel_spmd,
and reassembles.
"""
import sys

sys.path.insert(0, "/opt/trn_rl_repo")

import numpy as np
import ml_dtypes

import concourse.bass as bass
import concourse.mybir as mybir
import concourse.tile as tile
from concourse import bacc
from concourse.bass_utils import run_bass_kernel_spmd

# problem constants (hardcoded per spec)
N = 50000
D = 512
R = 4
E = 400000
NCORES = 8
ROWS_PER_CORE = N // NCORES          # 6250
NBLK = (ROWS_PER_CORE + 127) // 128  # 49
ROWS_PAD = NBLK * 128                # 6272
NK = 2 * R                           # 8 virtual relations
NF = D // 128                        # 4 feature pieces

BF16 = mybir.dt.bfloat16
F32 = mybir.dt.float32
I32 = mybir.dt.int32
PAD_ID = 255.0                       # selector id that never matches 0..127

_cache = {}


def _host_prep(features, w, bias, w_t, bias_t, edge_src, edge_dst):
    """Bucket edges, build per-core device arrays + the chunk schedule."""
    # per (k, core, block) edge buckets -> counts, then chunk schedule
    # big arrays: for each k, sorted-by-o edge list
    counts = np.zeros((NK, NCORES, NBLK), np.int64)
    per_k = []
    for k in range(NK):
        r, fwd = k // 2, (k % 2 == 0)
        o = np.asarray(edge_src[r] if fwd else edge_dst[r])
        g = np.asarray(edge_dst[r] if fwd else edge_src[r])
        order = np.argsort(o, kind="stable")
        o, g = o[order], g[order]
        core = o // ROWS_PER_CORE
        within = o % ROWS_PER_CORE
        blk = within // 128
        lid = within % 128
        # counts per (core, blk)
        cb = core * NBLK + blk
        counts[k] = np.bincount(cb, minlength=NCORES * NBLK).reshape(
            NCORES, NBLK)
        per_k.append((o, g, core, blk, lid, cb))

    # chunks per (k, b): max over cores, >= 1
    nchunks = np.maximum(
        1, (counts.max(axis=1) + 127) // 128)  # [NK, NBLK]
    # global column layout: iterate b, then k, then c
    col_of = np.zeros((NK, NBLK), np.int64)  # first column of (k,b)
    col = 0
    for b in range(NBLK):
        for k in range(NK):
            col_of[k, b] = col
            col += nchunks[k, b]
    ncol = col

    idx_all = np.zeros((NCORES, 128, ncol), np.int32)
    ids_all = np.full((NCORES, 128, ncol), PAD_ID, np.float32)
    deg = np.zeros((NCORES, NK, ROWS_PAD), np.float32)

    for k in range(NK):
        o, g, core, blk, lid, cb = per_k[k]
        # edges already sorted by o => sorted by (core, blk)
        # start offset of each (core, blk) bucket
        starts = np.zeros(NCORES * NBLK + 1, np.int64)
        np.cumsum(np.bincount(cb, minlength=NCORES * NBLK), out=starts[1:])
        for c in range(NCORES):
            deg[c, k, : ROWS_PER_CORE] = np.bincount(
                o[core == c] % ROWS_PER_CORE, minlength=ROWS_PER_CORE
            )[:ROWS_PER_CORE]
        # slot position for each edge within its bucket
        pos = np.arange(len(o)) - starts[cb]
        chunk = pos // 128
        slot = pos % 128
        gcol = col_of[k, blk] + chunk
        idx_all[core, slot, gcol] = g
        ids_all[core, slot, gcol] = lid

    full = deg.sum(axis=1)  # [NCORES, ROWS_PAD]
    inv = 1.0 / np.where(full == 0, 1.0, full)
    inv_deg = inv.reshape(NCORES, NBLK, 128).transpose(0, 2, 1).copy()

    # degT [NCORES, 8, ROWS_PAD] bf16 (counts exact in bf16 up to 256)
    degT = deg.astype(ml_dtypes.bfloat16)

    # weights: W_all[p, (k*4+f)*512 + o] = Wk[f*128+p, o]
    w = np.asarray(w, np.float32)
    w_t = np.asarray(w_t, np.float32)
    W_all = np.zeros((128, NK * NF * D), np.float32)
    biasmat = np.zeros((8, D), np.float32)
    for k in range(NK):
        r, fwd = k // 2, (k % 2 == 0)
        Wk = w[r] if fwd else w_t[r]
        for f in range(NF):
            W_all[:, (k * NF + f) * D:(k * NF + f + 1) * D] = \
                Wk[f * 128:(f + 1) * 128, :]
        biasmat[k] = (bias[r] if fwd else bias_t[r])
    W_all = W_all.astype(ml_dtypes.bfloat16)
    biasmat = biasmat.astype(ml_dtypes.bfloat16)

    x_bf = np.asarray(features, np.float32).astype(ml_dtypes.bfloat16)
    iota = np.tile(np.arange(128, dtype=np.float32), (128, 1))

    sched = {"nchunks": nchunks, "col_of": col_of, "ncol": ncol}
    in_maps = []
    for c in range(NCORES):
        in_maps.append({
            "xt": x_bf,
            "gidx": idx_all[c],
            "ids": ids_all[c],
            "wall": W_all,
            "biasmat": biasmat,
            "degT": degT[c],
            "invdeg": inv_deg[c].astype(np.float32),
            "iota": iota,
        })
    return sched, in_maps


def _build(sched, g_bufs=12, s_bufs=6, rep=1):
    nchunks, col_of = sched["nchunks"], sched["col_of"]
    ncol = sched["ncol"]

    nc = bacc.Bacc("TRN2", target_bir_lowering=False, debug=False,
                   num_devices=NCORES)
    xt = nc.dram_tensor("xt", [N, D], BF16, kind="ExternalInput").ap()
    gidx = nc.dram_tensor("gidx", [128, ncol], I32, kind="ExternalInput").ap()
    ids = nc.dram_tensor("ids", [128, ncol], F32, kind="ExternalInput").ap()
    wall = nc.dram_tensor("wall", [128, NK * NF * D], BF16,
                          kind="ExternalInput").ap()
    biasmat = nc.dram_tensor("biasmat", [8, D], BF16, kind="ExternalInput").ap()
    degT = nc.dram_tensor("degT", [8, ROWS_PAD], BF16,
                          kind="ExternalInput").ap()
    invdeg = nc.dram_tensor("invdeg", [128, NBLK], F32,
                            kind="ExternalInput").ap()
    iota = nc.dram_tensor("iota", [128, 128], F32, kind="ExternalInput").ap()
    out = nc.dram_tensor("out", [ROWS_PAD, D], F32, kind="ExternalOutput").ap()

    with tile.TileContext(nc) as tc:
        with (
            tc.tile_pool(name="const", bufs=1) as cp,
            tc.tile_pool(name="i1", bufs=16) as i1p,
            tc.tile_pool(name="g", bufs=g_bufs) as gp,
            tc.tile_pool(name="s", bufs=s_bufs) as sp,
            tc.tile_pool(name="axs", bufs=3) as axp,
            tc.tile_pool(name="os", bufs=2) as osp,
            tc.tile_pool(name="psax", bufs=2, space="PSUM") as psax,
            tc.tile_pool(name="psout", bufs=2, space="PSUM") as psout,
        ):
            # constants
            idx_t = cp.tile([128, ncol], I32)
            nc.sync.dma_start(out=idx_t[:], in_=gidx[:, :])
            ids_t = cp.tile([128, ncol], F32)
            nc.sync.dma_start(out=ids_t[:], in_=ids[:, :])
            w_t_ = cp.tile([128, NK * NF * D], BF16)
            nc.sync.dma_start(out=w_t_[:], in_=wall[:, :])
            bias_t_ = cp.tile([8, D], BF16)
            nc.sync.dma_start(out=bias_t_[:], in_=biasmat[:, :])
            degT_t = cp.tile([8, ROWS_PAD], BF16)
            nc.sync.dma_start(out=degT_t[:], in_=degT[:, :])
            inv_t = cp.tile([128, NBLK], F32)
            nc.sync.dma_start(out=inv_t[:], in_=invdeg[:, :])
            iota_t = cp.tile([128, 128], F32)
            nc.sync.dma_start(out=iota_t[:], in_=iota[:, :])

            for b in [bb for _ in range(rep) for bb in range(NBLK)]:
                outp = psout.tile([128, D], F32, space="PSUM")
                for k in range(NK):
                    axT = psax.tile([128, D], F32, space="PSUM")
                    nch = int(nchunks[k, b])
                    c0 = int(col_of[k, b])
                    for c in range(nch):
                        col = c0 + c
                        i1 = i1p.tile([128, 1], I32)
                        nc.scalar.copy(i1[:], idx_t[:, col:col + 1])
                        g = gp.tile([128, D], BF16)
                        nc.gpsimd.indirect_dma_start(
                            out=g[:], out_offset=None, in_=xt[:],
                            in_offset=bass.IndirectOffsetOnAxis(
                                ap=i1[:, :1], axis=0))
                        s = sp.tile([128, 128], BF16)
                        nc.vector.tensor_scalar(
                            out=s[:], in0=iota_t[:],
                            scalar1=ids_t[:, col:col + 1], scalar2=None,
                            op0=mybir.AluOpType.is_equal)
                        for f in range(NF):
                            nc.tensor.matmul(
                                axT[:, f * 128:(f + 1) * 128],
                                g[:, f * 128:(f + 1) * 128], s[:],
                                start=(c == 0 and f == 0),
                                stop=(c == nch - 1 and f == NF - 1))
                    ax_sb = axp.tile([128, D], BF16)
                    nc.vector.tensor_copy(ax_sb[:], axT[:])
                    for f in range(NF):
                        nc.tensor.matmul(
                            outp[:],
                            ax_sb[:, f * 128:(f + 1) * 128],
                            w_t_[:, (k * NF + f) * D:(k * NF + f + 1) * D],
                            start=(k == 0 and f == 0), stop=False)
                # deg * bias term
                nc.tensor.matmul(
                    outp[:], degT_t[:8, b * 128:(b + 1) * 128],
                    bias_t_[:8, :], start=False, stop=True)
                o_sb = osp.tile([128, D], F32)
                nc.vector.tensor_scalar(
                    out=o_sb[:], in0=outp[:], scalar1=inv_t[:, b:b + 1],
                    scalar2=None, op0=mybir.AluOpType.mult)
                nc.sync.dma_start(out=out[b * 128:(b + 1) * 128, :],
                                  in_=o_sb[:])
    nc.compile()
    return nc


def kernel(features, w, bias, w_t, bias_t, edge_src, edge_dst):
    sched, in_maps = _host_prep(features, w, bias, w_t, bias_t,
                                edge_src, edge_dst)
    key = ("v1", sched["ncol"],
           sched["nchunks"].tobytes(), sched["col_of"].tobytes())
    if key not in _cache:
        _cache.clear()
        _cache[key] = _build(sched)
    nc = _cache[key]
    res = run_bass_kernel_spmd(nc, in_maps, core_ids=list(range(NCORES)))
    parts = [res.results[c]["out"][:ROWS_PER_CORE] for c in range(NCORES)]
    return np.concatenate(parts, axis=0).astype(np.float32)
